# revision 39
# baseline (speedup 1.0000x reference)
"""GQA causal attention (RoPE) kernel for 8 TRN2 NeuronCores.

Sharding: core = b*4 + g  (b = batch 0..1, g = head-group 0..3).
Each core handles one batch element, 8 query heads (g*8..g*8+7) and the
2 KV heads (g*2, g*2+1) that serve them, plus the matching row-block of
Wo; per-core outputs are partial sums over the hidden dim that the host
reduces across the 4 groups of each batch.

On-core dataflow (all matmuls bf16 with f32 PSUM accumulation):
  QT = Wq_g.T @ X.T   [1024, 2048]   (feature-on-partition layout)
  KT = Wk_g.T @ X.T   [256, 2048]    + RoPE on QT/KT via a PE rotation
  VT = Wv_g.T @ X.T   -> PE-transposed to V [2048, 256]
  per head, per q-128 chunk: S[q,k] = QT_chunk.T x KT (causal-narrowed),
  additive -1e9 upper-tri mask on the diagonal 128-block, P = exp(S/sqrt(d))
  on ScalarE with fused row-sum (accum_out) -> per-row reciprocal ->
  P normalized in place; P blocks PE-transposed to P^T, ctx^T = V.T @ P^T;
  out_partial = ctx @ Wo_g (row block) accumulated over heads in PSUM.
"""

import os

import numpy as np
import ml_dtypes

import concourse.bass as bass
import concourse.mybir as mybir
import concourse.tile as tile
from concourse import bacc
from concourse.bass_utils import run_bass_kernel_spmd
from contextlib import ExitStack

B, S, H = 2, 2048, 4096
NH, NKV, HD = 32, 8, 128
BASE = 10000.0
N_CORES = 8
GROUPS = 4
NH_L = NH // GROUPS        # 8 local q heads
NKV_L = NKV // GROUPS      # 2 local kv heads
HC = H // 128              # 32 hidden chunks
TC = S // 128              # 16 token chunks
TB = S // 512              # 4 token 512-blocks
OC = H // 512              # 8 output-feature 512-blocks
SCALE = 1.0 / float(np.sqrt(HD))
NEG = -1e9

BF16 = mybir.dt.bfloat16
F32 = mybir.dt.float32
EXP = mybir.ActivationFunctionType.Exp
AX = mybir.AxisListType.X
ADD = mybir.AluOpType.add

_PROG = None
LAST_EXEC_NS = None
LAST_RESULTS = None


def _build():
    nc = bacc.Bacc(None, target_bir_lowering=False, debug=False)
    with tile.TileContext(nc) as tc:
        xt_d = nc.dram_tensor("xt", [128, HC, S], BF16, kind="ExternalInput")
        wq_d = nc.dram_tensor("wq", [NH_L, 128, HC, 128], BF16, kind="ExternalInput")
        wk_d = nc.dram_tensor("wk", [NKV_L, 128, HC, 128], BF16, kind="ExternalInput")
        wv_d = nc.dram_tensor("wv", [NKV_L, 128, HC, 128], BF16, kind="ExternalInput")
        wo_d = nc.dram_tensor("wo", [NH_L, 128, H], BF16, kind="ExternalInput")
        cos_d = nc.dram_tensor("cos", [128, S], BF16, kind="ExternalInput")
        sin_d = nc.dram_tensor("sin", [128, S], BF16, kind="ExternalInput")
        rt_d = nc.dram_tensor("rt", [128, 128], BF16, kind="ExternalInput")
        tria_d = nc.dram_tensor("tria", [128, 128], BF16, kind="ExternalInput")
        ident_d = nc.dram_tensor("ident", [128, 128], BF16, kind="ExternalInput")
        out_d = nc.dram_tensor("out_p", [S, H], F32, kind="ExternalOutput")

        with ExitStack() as stk:
            persist = stk.enter_context(tc.tile_pool(name="persist", bufs=1))
            q_all = persist.tile([128, NH_L, S], BF16, name="q_all", tag="q_all")
            k_all = persist.tile([128, NKV_L, S], BF16, name="k_all", tag="k_all")
            v_all = persist.tile([128, TC, NKV_L * 128], BF16, name="v_all", tag="v_all")
            rt_sb = persist.tile([128, 128], BF16, name="rt_sb", tag="rt_sb")
            tria_sb = persist.tile([128, 128], BF16, name="tria_sb", tag="tria_sb")
            ident_sb = persist.tile([128, 128], BF16, name="ident_sb", tag="ident_sb")
            bias0 = persist.tile([128, 1], F32, name="bias0", tag="bias0")

            nc.sync.dma_start(out=rt_sb[:], in_=rt_d[:])
            nc.sync.dma_start(out=tria_sb[:], in_=tria_d[:])
            nc.sync.dma_start(out=ident_sb[:], in_=ident_d[:])
            nc.any.memset(bias0[:], 0.0)

            # one PSUM pool for the whole program:
            #   tag "acc" (5 banks): projection accumulators, attention AV,
            #                        o-proj accumulators
            #   tag "sp"  (3 banks): rot matmuls, QK scores
            ps_pool = stk.enter_context(tc.tile_pool(name="ps", bufs=1, space="PSUM"))

            def acc_tile():
                return ps_pool.tile([128, 512], F32, name="acc", tag="acc", bufs=4)

            def sp_tile(dt=F32):
                # [128, 1024] row tiles (2 banks): two 512 k-blocks share one
                # tile so the exp runs as a single wide ScalarE instruction
                return ps_pool.tile([128, 1024], dt, name="spt", tag="sp", bufs=2)

            # ---------------- projections ----------------
            with ExitStack() as proj:
                cs_pool = proj.enter_context(tc.tile_pool(name="csp", bufs=1))
                cos_sb = cs_pool.tile([128, S], BF16, name="cos_sb", tag="cos_sb")
                sin_sb = cs_pool.tile([128, S], BF16, name="sin_sb", tag="sin_sb")
                nc.sync.dma_start(out=cos_sb[:], in_=cos_d[:])
                nc.sync.dma_start(out=sin_sb[:], in_=sin_d[:])
                wpool = proj.enter_context(tc.tile_pool(name="wpool", bufs=4))
                raw_pool = proj.enter_context(tc.tile_pool(name="rawp", bufs=1))
                tmp_pool = proj.enter_context(tc.tile_pool(name="tmpp", bufs=2))

                def load_w_quarters(w_d, f):
                    ws = []
                    for qtr in range(4):
                        wt = wpool.tile([128, 8, 128], BF16, name="wt", tag="wt")
                        nc.sync.dma_start(out=wt[:], in_=w_d[f, :, qtr * 8:(qtr + 1) * 8, :])
                        ws.append(wt)
                    return ws

                # first projection's weights before the bulk xt DMAs, so the
                # PE can start as soon as the first xt tile lands
                ws_v0 = load_w_quarters(wv_d, 0)

                xt_pool = proj.enter_context(tc.tile_pool(name="xtp", bufs=1))
                xts = []
                for i in range(16):
                    t = xt_pool.tile([128, 2, S], BF16, name=f"xtt{i}", tag=f"xtt{i}")
                    nc.sync.dma_start(out=t[:], in_=xt_d[:, i * 2:(i + 1) * 2, :])
                    xts.append(t)

                def xt_ap(hc, lo, hi):
                    return xts[hc // 2][:, hc % 2, lo:hi]

                def project_T(w_d, f, ws=None):
                    if ws is None:
                        ws = load_w_quarters(w_d, f)
                    pss = [acc_tile() for _ in range(TB)]
                    for hc in range(HC):
                        lhsT = ws[hc // 8][:, hc % 8, :]
                        for tb in range(TB):
                            nc.tensor.matmul(
                                pss[tb][:], lhsT, xt_ap(hc, tb * 512, (tb + 1) * 512),
                                start=(hc == 0), stop=(hc == HC - 1),
                            )
                    return pss

                def rope_into(pss, dst, idx):
                    raw = raw_pool.tile([128, S], BF16, name="raw", tag="raw")
                    for tb in range(TB):
                        nc.scalar.copy(raw[:, tb * 512:(tb + 1) * 512], pss[tb][:])
                    for tb in range(TB):
                        sl = slice(tb * 512, (tb + 1) * 512)
                        rps = sp_tile()
                        nc.tensor.matmul(rps[:, :512], rt_sb[:], raw[:, sl], start=True, stop=True)
                        t1 = tmp_pool.tile([128, 512], F32, name="t1", tag="t1")
                        t2 = tmp_pool.tile([128, 512], F32, name="t2", tag="t2")
                        nc.vector.tensor_mul(t1[:], raw[:, sl], cos_sb[:, sl])
                        nc.vector.tensor_mul(t2[:], rps[:, :512], sin_sb[:, sl])
                        nc.vector.tensor_add(dst[:, idx, sl], t1[:], t2[:])

                for f in range(NKV_L):
                    pss = project_T(wv_d, f, ws_v0 if f == 0 else None)
                    raw = raw_pool.tile([128, S], BF16, name="raw", tag="raw")
                    for tb in range(TB):
                        nc.scalar.copy(raw[:, tb * 512:(tb + 1) * 512], pss[tb][:])
                    nc.sync.dma_start_transpose(
                        out=v_all[:, :, f * 128:(f + 1) * 128], in_=raw[:],
                    )
                for f in range(NKV_L):
                    rope_into(project_T(wk_d, f), k_all, f)
                for f in range(NH_L):
                    rope_into(project_T(wq_d, f), q_all, f)

            # ---------------- attention + output projection ----------------
            with ExitStack() as att:
                wo_pool = att.enter_context(tc.tile_pool(name="wop", bufs=1))
                wo_sb = wo_pool.tile([128, NH_L, H], BF16, name="wo_sb", tag="wo_sb")
                for h in range(NH_L):
                    # SWDGE queues: keep the HWDGE queues free for the
                    # latency-critical P^T transposes
                    nc.gpsimd.dma_start(out=wo_sb[:, h, :], in_=wo_d[h])

                ct_pool = att.enter_context(tc.tile_pool(name="ctp", bufs=3))
                rs_pool = att.enter_context(tc.tile_pool(name="rsp", bufs=8))
                osb_pool = att.enter_context(tc.tile_pool(name="osbp", bufs=4))

                p_pool = att.enter_context(tc.tile_pool(name="pp", bufs=5))
                ptt_pool = att.enter_context(tc.tile_pool(name="pttp", bufs=2))

                cts_by_qb = {}

                def softmax_part(qb, h, drip=None):
                    """QK + mask + exp + row-normalize + P^T transpose DMAs."""
                    kv = h // (NH_L // NKV_L)
                    ptt = ptt_pool.tile([128, 16, 4, 128], BF16, name="ptt", tag="ptt")
                    for qcl in range(4):
                        qg = 4 * qb + qcl
                        Wq = (qg + 1) * 128
                        nkb = qb + 1
                        p_sb = p_pool.tile([128, S], BF16, name="p_sb", tag="p")
                        rs = rs_pool.tile([128, 2], F32, name="rs", tag="rs")
                        for kb2 in range(0, nkb, 2):
                            sp = sp_tile()
                            w0 = 0
                            for j in (0, 1):
                                kb = kb2 + j
                                if kb >= nkb:
                                    break
                                wk_ = 512 if kb < qb else Wq - kb * 512
                                nc.tensor.matmul(
                                    sp[:, j * 512:j * 512 + wk_],
                                    q_all[:, h, qg * 128:(qg + 1) * 128],
                                    k_all[:, kv, kb * 512:kb * 512 + wk_],
                                    start=True, stop=(kb != qb),
                                )
                                if kb == qb:
                                    nc.tensor.matmul(
                                        sp[:, j * 512 + wk_ - 128:j * 512 + wk_],
                                        ident_sb[:], tria_sb[:],
                                        start=False, stop=True,
                                    )
                                w0 += wk_
                            nc.scalar.activation(
                                p_sb[:, kb2 * 512:kb2 * 512 + w0], sp[:, :w0], EXP,
                                bias=bias0[:], scale=SCALE,
                                accum_out=rs[:, kb2 // 2:kb2 // 2 + 1],
                            )
                        dsum = rs_pool.tile([128, 1], F32, name="dsum", tag="dsum")
                        nc.vector.tensor_reduce(
                            dsum[:], rs[:, :(nkb + 1) // 2], axis=AX, op=ADD)
                        rq = rs_pool.tile([128, 1], F32, name="rq", tag="rq")
                        nc.vector.reciprocal(rq[:], dsum[:])
                        nc.vector.tensor_scalar_mul(p_sb[:, :Wq], p_sb[:, :Wq], rq[:])
                        nc.sync.dma_start_transpose(
                            out=ptt[:, :Wq // 128, qcl, :], in_=p_sb[:, :Wq],
                        )
                        if drip is not None:
                            drip()
                    return ptt

                def av_part(qb, h, ptt):
                    kv = h // (NH_L // NKV_L)
                    nkc = 4 * (qb + 1)
                    av = acc_tile()
                    for kc in range(nkc):
                        d = max(0, kc - 4 * qb)
                        off = d * 128
                        nc.tensor.matmul(
                            av[:, off:512],
                            v_all[:, kc, kv * 128:(kv + 1) * 128],
                            ptt[:, kc, d:4, :],
                            start=(kc == 0), stop=(kc == nkc - 1),
                        )
                    nc.vector.tensor_copy(cts_by_qb[qb][:, h, :], av[:])

                def oproj_group(qb, qcl, oc):
                    cts = cts_by_qb[qb]
                    qc = qb * 4 + qcl
                    op = acc_tile()
                    for h in range(NH_L):
                        nc.tensor.matmul(
                            op[:],
                            cts[:, h, qcl * 128:(qcl + 1) * 128],
                            wo_sb[:, h, oc * 512:(oc + 1) * 512],
                            start=(h == 0), stop=(h == NH_L - 1),
                        )
                    osb = osb_pool.tile([128, 512], F32, name="osb", tag="osb")
                    nc.vector.tensor_copy(osb[:], op[:])
                    nc.gpsimd.dma_start(
                        out=out_d[qc * 128:(qc + 1) * 128, oc * 512:(oc + 1) * 512],
                        in_=osb[:],
                    )

                # Software-pipelined: the next head's QK matmuls are emitted
                # ahead of this head's AV in the PE stream, so the PE never
                # waits for the exp -> normalize -> transpose chain.  The
                # previous block's o-projection groups are drip-fed between
                # head iterations so ScalarE always has fresh scores to exp
                # while the PE runs dense o-proj/AV work.
                pairs = [(qb, h) for qb in (0, 3, 2, 1) for h in range(NH_L)]
                prev = None
                for qb, h in pairs:
                    if h == 0:
                        cts_by_qb[qb] = ct_pool.tile(
                            [128, NH_L, 512], BF16, name="cts", tag="ct")
                    ptt = softmax_part(qb, h)
                    if prev is not None:
                        pqb, ph, pptt = prev
                        av_part(pqb, ph, pptt)
                        if ph == NH_L - 1:
                            for qcl in range(4):
                                for oc in range(OC):
                                    oproj_group(pqb, qcl, oc)
                            cts_by_qb.pop(pqb)
                    prev = (qb, h, ptt)
                pqb, ph, pptt = prev
                av_part(pqb, ph, pptt)
                for qcl in range(4):
                    for oc in range(OC):
                        oproj_group(pqb, qcl, oc)
                cts_by_qb.pop(pqb)
    nc.compile()
    return nc


def _prep_inputs(hidden_states, position_ids, Wq, Wk, Wv, Wo):
    bf = ml_dtypes.bfloat16
    hidden_states = np.asarray(hidden_states, dtype=np.float32)
    position_ids = np.asarray(position_ids)
    Wq = np.asarray(Wq, dtype=np.float32)
    Wk = np.asarray(Wk, dtype=np.float32)
    Wv = np.asarray(Wv, dtype=np.float32)
    Wo = np.asarray(Wo, dtype=np.float32)

    inv_freq = (1.0 / (BASE ** (np.arange(0, HD, 2, dtype=np.float32) / HD))).astype(np.float32)
    rt = np.zeros((128, 128), dtype=np.float32)
    rt[np.arange(64, 128), np.arange(0, 64)] = -1.0
    rt[np.arange(0, 64), np.arange(64, 128)] = 1.0
    rt = rt.astype(bf)
    ident = np.eye(128, dtype=np.float32).astype(bf)
    ii = np.arange(128)
    tria = np.where(ii[None, :] > ii[:, None], np.float32(NEG), np.float32(0.0)).astype(bf)

    per_batch = []
    for b in range(B):
        xt = np.ascontiguousarray(
            hidden_states[b].T.reshape(HC, 128, S).transpose(1, 0, 2)
        ).astype(bf)
        pos = position_ids[b].astype(np.float32)
        freqs = pos[:, None] * inv_freq[None, :]           # [S, 64]
        emb = np.concatenate([freqs, freqs], axis=1)       # [S, 128]
        cos = np.ascontiguousarray(np.cos(emb).T).astype(bf)
        sin = np.ascontiguousarray(np.sin(emb).T).astype(bf)
        per_batch.append((xt, cos, sin))

    in_maps = []
    for core in range(N_CORES):
        b, g = core // GROUPS, core % GROUPS
        xt, cos, sin = per_batch[b]
        wq = np.ascontiguousarray(
            Wq[:, g * NH_L * HD:(g + 1) * NH_L * HD]
            .reshape(HC, 128, NH_L, 128).transpose(2, 1, 0, 3)
        ).astype(bf)
        wk = np.ascontiguousarray(
            Wk[:, g * NKV_L * HD:(g + 1) * NKV_L * HD]
            .reshape(HC, 128, NKV_L, 128).transpose(2, 1, 0, 3)
        ).astype(bf)
        wv = np.ascontiguousarray(
            Wv[:, g * NKV_L * HD:(g + 1) * NKV_L * HD]
            .reshape(HC, 128, NKV_L, 128).transpose(2, 1, 0, 3)
        ).astype(bf)
        wo = np.ascontiguousarray(
            Wo[g * NH_L * HD:(g + 1) * NH_L * HD, :].reshape(NH_L, 128, H)
        ).astype(bf)
        in_maps.append({
            "xt": xt, "wq": wq, "wk": wk, "wv": wv, "wo": wo,
            "cos": cos, "sin": sin, "rt": rt, "tria": tria, "ident": ident,
        })
    return in_maps


def kernel(hidden_states, position_ids, Wq, Wk, Wv, Wo):
    global _PROG, LAST_EXEC_NS, LAST_RESULTS
    if _PROG is None:
        _PROG = _build()
    nc = _PROG
    in_maps = _prep_inputs(hidden_states, position_ids, Wq, Wk, Wv, Wo)
    trace = os.environ.get("BASS_KERNEL_TRACE", "0") == "1"
    res = run_bass_kernel_spmd(nc, in_maps, core_ids=list(range(N_CORES)), trace=trace)
    LAST_EXEC_NS = res.exec_time_ns
    LAST_RESULTS = res
    out = np.zeros((B, S, H), dtype=np.float32)
    for core in range(N_CORES):
        out[core // GROUPS] += res.results[core]["out_p"]
    return out
